# revision 9
# baseline (speedup 1.0000x reference)
"""DSAttention Trainium2 kernel.

Reference computation (per batch b, head h):
    S[q,s]  = (Q[q]·K[s]) * tau[b] + delta[b,s]
    S      += causal mask (s > q -> -inf)
    A       = softmax(S / sqrt(E), axis=s)
    O[q,:]  = sum_s A[q,s] * V[s,:]

Shapes: B=2, L=2048, H=16, E=64 -> 32 (b,h) pairs, 4 per NeuronCore x 8 cores.

Device strategy (per core, per head):
  - Q^T, K^T resident in SBUF as [e=64, L] (host pre-transposed).
  - V with a ones-column appended: [128, 16 chunks, 65]; the AV matmul then
    produces both O^T (rows 0..63) and the softmax denominator (row 64) in
    one PSUM accumulation.
  - Loop over s-chunks n (128 keys each): S^T[s,q] = K_n^T.T @ Q^T computed
    only for q >= 128n (causal skip), as <=512-wide matmul pieces into a
    [128,1024] PSUM tile (2 halves of the q range, double buffered).
  - One Exp activation per (chunk, half): exp(dot * (tau/8) + delta/8) with
    tau as a per-partition scale AP and delta/8 as a per-partition bias AP.
    No max-subtraction: |dot*tau/8 + delta/8| <= ~8, exp is safe in fp32.
  - Diagonal 128x128 block masked by multiplying with an upper-triangular
    0/1 matrix after the exp.
  - AV: O^T[65, q] += V1_n.T @ A^T accumulated over chunks in a [65, 2048]
    PSUM tile; when a 512-column q-tile is complete, divide rows 0..63 by
    row 64 (reciprocal + partition-broadcast DMA + scalar_tensor_tensor)
    and DMA out.  Output is O^T [64, L] per head; host transposes back.
  - Matmuls run in float32r (full-rate fp32 PE mode).
"""

import sys

sys.path.insert(0, "/opt/trn_rl_repo")

import ml_dtypes
import numpy as np

import concourse.bass as bass
import concourse.tile as tile
from concourse import bacc, mybir
from concourse.masks import make_identity, make_upper_triangular

B, L, H, E = 2, 2048, 16, 64
NCORES = 8
HPC = (B * H) // NCORES  # heads per core = 4
NCH = L // 128  # s-chunks per head = 16
SCALE = 1.0 / 8.0  # 1/sqrt(E)
F32 = mybir.dt.float32
F32R = mybir.dt.float32r
BF16 = mybir.dt.bfloat16
EXP = mybir.ActivationFunctionType.Exp
MULT = mybir.AluOpType.mult


def _pieces(n):
    """512-aligned matmul pieces covering q in [128n, L)."""
    q0 = 128 * n
    out = []
    c = q0
    w = 512 - 128 * (n % 4)
    out.append((c, w))
    c += w
    while c < L:
        out.append((c, 512))
        c += 512
    return out


def _body(tc, qT, kT, v1, dlt, tau_in, out):
    nc = tc.nc
    from contextlib import ExitStack

    with ExitStack() as ctx:
        const = ctx.enter_context(tc.tile_pool(name="const", bufs=1))
        qk_pool = ctx.enter_context(tc.tile_pool(name="qk", bufs=2))
        v_pool = ctx.enter_context(tc.tile_pool(name="v", bufs=2))
        hd_pool = ctx.enter_context(tc.tile_pool(name="hd", bufs=2))
        a_pool = ctx.enter_context(tc.tile_pool(name="a", bufs=3))
        o_pool = ctx.enter_context(tc.tile_pool(name="o", bufs=2))
        r_pool = ctx.enter_context(tc.tile_pool(name="r", bufs=2))
        ps_pool = ctx.enter_context(tc.tile_pool(name="psS", bufs=2, space="PSUM"))
        po_pool = ctx.enter_context(tc.tile_pool(name="psO", bufs=2, space="PSUM"))

        trimask = const.tile([128, 128], BF16, name="trimask")
        make_upper_triangular(nc, trimask[:], val=1.0, diag=True)
        ident65 = const.tile([65, 65], F32, name="ident65")
        make_identity(nc, ident65[:])

        for i in range(HPC):
            qt = qk_pool.tile([64, L], BF16, tag="qt", name=f"qt{i}")
            nc.sync.dma_start(qt[:], qT[i])
            kt = qk_pool.tile([64, L], BF16, tag="kt", name=f"kt{i}")
            nc.sync.dma_start(kt[:], kT[i])
            vt = v_pool.tile([128, NCH * 65], BF16, tag="vt", name=f"vt{i}")
            nc.sync.dma_start(vt[:], v1[i])
            dl = hd_pool.tile([128, NCH], F32, tag="dl", name=f"dl{i}")
            nc.sync.dma_start(dl[:], dlt[i])
            dls = hd_pool.tile([128, NCH], F32, tag="dls", name=f"dls{i}")
            nc.vector.tensor_scalar_mul(dls[:], dl[:], SCALE)
            tt = hd_pool.tile([128, 1], F32, tag="tt", name=f"tt{i}")
            nc.sync.dma_start(tt[:], tau_in[i : i + 1, 0:1].to_broadcast([128, 1]))
            tts = hd_pool.tile([128, 1], F32, tag="tts", name=f"tts{i}")
            nc.vector.tensor_scalar_mul(tts[:], tt[:], SCALE)

            # Two q-window phases per head: oT is [65, 1024] (2 PSUM
            # banks), leaving room to double-buffer both psumS and the
            # oT/psT slot.  AV matmuls are emitted two units behind their
            # QK/exp so the in-order PE queue never stalls on ACT.
            for phase in range(2):
                qlo = 1024 * phase
                qhi = qlo + 1024
                oT = po_pool.tile([65, 1024], F32, tag="oT", name=f"oT{i}_{phase}")
                units = []  # (n, pieces, a_sb)
                pend = []   # emitted QK/exp awaiting AV emission
                nlist = list(range(qhi // 128))

                def emit_av(u):
                    n, pieces, a_sb = u
                    for c0, w in pieces:
                        b = (c0 - qlo) // 512
                        j = 2 * phase + b
                        nc.tensor.matmul(
                            oT[:, c0 - qlo : c0 - qlo + w],
                            lhsT=vt[:, n * 65 : n * 65 + 65],
                            rhs=a_sb[:, c0 - qlo : c0 - qlo + w],
                            start=(n == 0),
                            stop=(n == 4 * j + 3),
                        )

                for n in nlist:
                    q0 = max(128 * n, qlo)
                    pieces = []
                    c = q0
                    w0 = 512 * (q0 // 512 + 1) - q0
                    pieces.append((c, w0))
                    c += w0
                    while c < qhi:
                        pieces.append((c, 512))
                        c += 512
                    ps = ps_pool.tile([128, 1024], F32, tag="ps", name=f"ps{i}_{phase}_{n}")
                    for c0, w in pieces:
                        nc.tensor.matmul(
                            ps[:, c0 - qlo : c0 - qlo + w],
                            lhsT=kt[:, 128 * n : 128 * n + 128],
                            rhs=qt[:, c0 : c0 + w],
                            start=True,
                            stop=True,
                        )
                    a_sb = a_pool.tile([128, 1024], BF16, tag="a", name=f"a{i}_{phase}_{n}")
                    nc.scalar.activation(
                        a_sb[:, q0 - qlo : 1024],
                        ps[:, q0 - qlo : 1024],
                        EXP,
                        bias=dls[:, n : n + 1],
                        scale=tts[:, 0:1],
                    )
                    if 128 * n >= qlo:
                        nc.vector.tensor_mul(
                            a_sb[:, q0 - qlo : q0 - qlo + 128],
                            a_sb[:, q0 - qlo : q0 - qlo + 128],
                            trimask[:],
                        )
                    pend.append((n, pieces, a_sb))
                    if len(pend) > 2:
                        emit_av(pend.pop(0))
                for u in pend:
                    emit_av(u)

                # Window finalization: PSUM -> SBUF, PE-transpose each 128-q
                # chunk, reciprocal of the denominator row, scale, store.
                o_sb = o_pool.tile([65, 1024], F32, tag="osb", name=f"osb{i}_{phase}")
                nc.vector.tensor_copy(o_sb[:], oT[:, :])
                psT = po_pool.tile([128, 1024], F32, tag="oT", name=f"psT{i}_{phase}")
                for t in range(8):
                    nc.tensor.transpose(
                        psT[:, 128 * t : 128 * t + 65],
                        o_sb[:, 128 * t : 128 * t + 128],
                        ident65[:],
                    )
                psT3 = psT.rearrange("p (t c) -> p t c", c=128)
                recipv = r_pool.tile([128, 8], F32, tag="rr", name=f"rr{i}_{phase}")
                nc.vector.reciprocal(recipv[:], psT3[:, :, 64])
                o2 = o_pool.tile([128, 8, 64], F32, tag="o2", name=f"o2_{i}_{phase}")
                nc.vector.scalar_tensor_tensor(
                    out=o2[:],
                    in0=psT3[:, :, 0:64],
                    scalar=1.0,
                    in1=recipv[:, :, None].to_broadcast([128, 8, 64]),
                    op0=MULT,
                    op1=MULT,
                )
                nc.sync.dma_start(
                    out[i].rearrange("(w t p) d -> w p t d", p=128, t=8)[phase],
                    o2[:],
                )


_CACHED = None


def _build():
    global _CACHED
    if _CACHED is not None:
        return _CACHED
    nc = bacc.Bacc("TRN2", target_bir_lowering=False, debug=False)
    qT = nc.dram_tensor("qT", [HPC, 64, L], BF16, kind="ExternalInput").ap()
    kT = nc.dram_tensor("kT", [HPC, 64, L], BF16, kind="ExternalInput").ap()
    v1 = nc.dram_tensor("v1", [HPC, 128, NCH * 65], BF16, kind="ExternalInput").ap()
    dlt = nc.dram_tensor("dlt", [HPC, 128, NCH], F32, kind="ExternalInput").ap()
    tau_in = nc.dram_tensor("tau_in", [HPC, 1], F32, kind="ExternalInput").ap()
    out = nc.dram_tensor("out", [HPC, L, E], F32, kind="ExternalOutput").ap()
    with tile.TileContext(nc) as tc:
        _body(tc, qT, kT, v1, dlt, tau_in, out)
    nc.compile()
    _CACHED = nc
    return nc


def _prep_in_maps(queries, keys, values, tau, delta):
    """Shard + relayout the full inputs into 8 per-core input dicts."""
    queries = np.asarray(queries, dtype=np.float32)
    keys = np.asarray(keys, dtype=np.float32)
    values = np.asarray(values, dtype=np.float32)
    tau = np.asarray(tau, dtype=np.float32)
    delta = np.asarray(delta, dtype=np.float32)

    in_maps = []
    for core in range(NCORES):
        qTs = np.empty((HPC, 64, L), ml_dtypes.bfloat16)
        kTs = np.empty((HPC, 64, L), ml_dtypes.bfloat16)
        v1s = np.empty((HPC, 128, NCH * 65), ml_dtypes.bfloat16)
        dls = np.empty((HPC, 128, NCH), np.float32)
        tas = np.empty((HPC, 1), np.float32)
        for slot in range(HPC):
            g = core * HPC + slot
            b, h = divmod(g, H)
            qTs[slot] = queries[b, :, h, :].T
            kTs[slot] = keys[b, :, h, :].T
            v = values[b, :, h, :].reshape(NCH, 128, E).transpose(1, 0, 2)
            vv = np.concatenate([v, np.ones((128, NCH, 1), np.float32)], axis=2)
            v1s[slot] = vv.reshape(128, NCH * 65).astype(ml_dtypes.bfloat16)
            dls[slot] = delta[b].reshape(NCH, 128).T
            tas[slot, 0] = tau[b, 0]
        in_maps.append(
            {"qT": qTs, "kT": kTs, "v1": v1s, "dlt": dls, "tau_in": tas}
        )
    return in_maps


def _assemble(results):
    O = np.empty((B, L, H, E), np.float32)
    for core in range(NCORES):
        o = results[core]["out"]  # [HPC, L, E]
        for slot in range(HPC):
            g = core * HPC + slot
            b, h = divmod(g, H)
            O[b, :, h, :] = o[slot]
    return O


def run(inputs, trace=False, **kwargs):
    from concourse import bass_utils

    nc = _build()
    in_maps = _prep_in_maps(**inputs)
    res = bass_utils.run_bass_kernel_spmd(
        nc, in_maps, core_ids=list(range(NCORES)), trace=trace, **kwargs
    )
    return _assemble(res.results), res


def kernel(**inputs):
    return run(inputs, trace=False)[0]


# revision 10
# speedup vs baseline: 1.5476x; 1.5476x over previous
"""DSAttention Trainium2 kernel.

Reference computation (per batch b, head h):
    S[q,s]  = (Q[q]·K[s]) * tau[b] + delta[b,s]
    S      += causal mask (s > q -> -inf)
    A       = softmax(S / sqrt(E), axis=s)
    O[q,:]  = sum_s A[q,s] * V[s,:]

Shapes: B=2, L=2048, H=16, E=64 -> 32 (b,h) pairs, 4 per NeuronCore x 8 cores.

Device strategy (per core, per head):
  - Q^T, K^T resident in SBUF as [e=64, L] (host pre-transposed).
  - V with a ones-column appended: [128, 16 chunks, 65]; the AV matmul then
    produces both O^T (rows 0..63) and the softmax denominator (row 64) in
    one PSUM accumulation.
  - Loop over s-chunks n (128 keys each): S^T[s,q] = K_n^T.T @ Q^T computed
    only for q >= 128n (causal skip), as <=512-wide matmul pieces into a
    [128,1024] PSUM tile (2 halves of the q range, double buffered).
  - One Exp activation per (chunk, half): exp(dot * (tau/8) + delta/8) with
    tau as a per-partition scale AP and delta/8 as a per-partition bias AP.
    No max-subtraction: |dot*tau/8 + delta/8| <= ~8, exp is safe in fp32.
  - Diagonal 128x128 block masked by multiplying with an upper-triangular
    0/1 matrix after the exp.
  - AV: O^T[65, q] += V1_n.T @ A^T accumulated over chunks in a [65, 2048]
    PSUM tile; when a 512-column q-tile is complete, divide rows 0..63 by
    row 64 (reciprocal + partition-broadcast DMA + scalar_tensor_tensor)
    and DMA out.  Output is O^T [64, L] per head; host transposes back.
  - Matmuls run in float32r (full-rate fp32 PE mode).
"""

import sys

sys.path.insert(0, "/opt/trn_rl_repo")

import ml_dtypes
import numpy as np

import concourse.bass as bass
import concourse.tile as tile
from concourse import bacc, mybir
from concourse.masks import make_identity, make_upper_triangular

B, L, H, E = 2, 2048, 16, 64
NCORES = 8
HPC = (B * H) // NCORES  # heads per core = 4
NCH = L // 128  # s-chunks per head = 16
SCALE = 1.0 / 8.0  # 1/sqrt(E)
F32 = mybir.dt.float32
F32R = mybir.dt.float32r
BF16 = mybir.dt.bfloat16
EXP = mybir.ActivationFunctionType.Exp
MULT = mybir.AluOpType.mult


def _pieces(n):
    """512-aligned matmul pieces covering q in [128n, L)."""
    q0 = 128 * n
    out = []
    c = q0
    w = 512 - 128 * (n % 4)
    out.append((c, w))
    c += w
    while c < L:
        out.append((c, 512))
        c += 512
    return out


def _body(tc, qT, kT, v1, dlt, tau_in, out):
    nc = tc.nc
    from contextlib import ExitStack

    with ExitStack() as ctx:
        const = ctx.enter_context(tc.tile_pool(name="const", bufs=1))
        qk_pool = ctx.enter_context(tc.tile_pool(name="qk", bufs=2))
        v_pool = ctx.enter_context(tc.tile_pool(name="v", bufs=2))
        hd_pool = ctx.enter_context(tc.tile_pool(name="hd", bufs=2))
        a_pool = ctx.enter_context(tc.tile_pool(name="a", bufs=3))
        o_pool = ctx.enter_context(tc.tile_pool(name="o", bufs=2))
        r_pool = ctx.enter_context(tc.tile_pool(name="r", bufs=2))
        ps_pool = ctx.enter_context(tc.tile_pool(name="psS", bufs=2, space="PSUM"))
        po_pool = ctx.enter_context(tc.tile_pool(name="psO", bufs=2, space="PSUM"))

        trimask = const.tile([128, 128], BF16, name="trimask")
        make_upper_triangular(nc, trimask[:], val=1.0, diag=True)
        ident65 = const.tile([65, 65], F32, name="ident65")
        make_identity(nc, ident65[:])

        for i in range(HPC):
            qt = qk_pool.tile([128, L], BF16, tag="qt", name=f"qt{i}")
            nc.sync.dma_start(qt[:], qT[i])
            kt = qk_pool.tile([128, L], BF16, tag="kt", name=f"kt{i}")
            nc.sync.dma_start(kt[:], kT[i])
            vt = v_pool.tile([128, NCH * 65], BF16, tag="vt", name=f"vt{i}")
            nc.sync.dma_start(vt[:], v1[i])
            dl = hd_pool.tile([128, NCH], F32, tag="dl", name=f"dl{i}")
            nc.sync.dma_start(dl[:], dlt[i])
            dls = hd_pool.tile([128, NCH], F32, tag="dls", name=f"dls{i}")
            nc.vector.tensor_scalar_mul(dls[:], dl[:], SCALE)
            tt = hd_pool.tile([128, 1], F32, tag="tt", name=f"tt{i}")
            nc.sync.dma_start(tt[:], tau_in[i : i + 1, 0:1].to_broadcast([128, 1]))
            tts = hd_pool.tile([128, 1], F32, tag="tts", name=f"tts{i}")
            nc.vector.tensor_scalar_mul(tts[:], tt[:], SCALE)

            # Two q-window phases per head: oT is [65, 1024] (2 PSUM
            # banks), leaving room to double-buffer both psumS and the
            # oT/psT slot.  AV matmuls are emitted two units behind their
            # QK/exp so the in-order PE queue never stalls on ACT.
            for phase in range(2):
                qlo = 1024 * phase
                qhi = qlo + 1024
                oT = po_pool.tile([65, 1024], F32, tag="oT", name=f"oT{i}_{phase}")
                units = []  # (n, pieces, a_sb)
                pend = []   # emitted QK/exp awaiting AV emission
                nlist = list(range(qhi // 128))

                def emit_av(u):
                    n, pieces, a_sb = u
                    for c0, w in pieces:
                        b = (c0 - qlo) // 512
                        j = 2 * phase + b
                        nc.tensor.matmul(
                            oT[:, c0 - qlo : c0 - qlo + w],
                            lhsT=vt[:, n * 65 : n * 65 + 65],
                            rhs=a_sb[:, c0 - qlo : c0 - qlo + w],
                            start=(n == 0),
                            stop=(n == 4 * j + 3),
                        )

                for n in nlist:
                    q0 = max(128 * n, qlo)
                    pieces = []
                    c = q0
                    w0 = 512 * (q0 // 512 + 1) - q0
                    pieces.append((c, w0))
                    c += w0
                    while c < qhi:
                        pieces.append((c, 512))
                        c += 512
                    ps = ps_pool.tile([128, 1024], F32, tag="ps", name=f"ps{i}_{phase}_{n}")
                    for c0, w in pieces:
                        nc.tensor.matmul(
                            ps[:, c0 - qlo : c0 - qlo + w],
                            lhsT=kt[:, 128 * n : 128 * n + 128],
                            rhs=qt[:, c0 : c0 + w],
                            start=True,
                            stop=True,
                        )
                    a_sb = a_pool.tile([128, 1024], BF16, tag="a", name=f"a{i}_{phase}_{n}")
                    nc.scalar.activation(
                        a_sb[:, q0 - qlo : 1024],
                        ps[:, q0 - qlo : 1024],
                        EXP,
                        bias=dls[:, n : n + 1],
                        scale=tts[:, 0:1],
                    )
                    if 128 * n >= qlo:
                        nc.vector.tensor_mul(
                            a_sb[:, q0 - qlo : q0 - qlo + 128],
                            a_sb[:, q0 - qlo : q0 - qlo + 128],
                            trimask[:],
                        )
                    pend.append((n, pieces, a_sb))
                    if len(pend) > 2:
                        emit_av(pend.pop(0))
                for u in pend:
                    emit_av(u)

                # Window finalization: PSUM -> SBUF, PE-transpose each 128-q
                # chunk, reciprocal of the denominator row, scale, store.
                o_sb = o_pool.tile([65, 1024], F32, tag="osb", name=f"osb{i}_{phase}")
                nc.vector.tensor_copy(o_sb[:], oT[:, :])
                psT = po_pool.tile([128, 1024], F32, tag="oT", name=f"psT{i}_{phase}")
                for t in range(8):
                    nc.tensor.transpose(
                        psT[:, 128 * t : 128 * t + 65],
                        o_sb[:, 128 * t : 128 * t + 128],
                        ident65[:],
                    )
                psT3 = psT.rearrange("p (t c) -> p t c", c=128)
                recipv = r_pool.tile([128, 8], F32, tag="rr", name=f"rr{i}_{phase}")
                nc.vector.reciprocal(recipv[:], psT3[:, :, 64])
                o2 = o_pool.tile([128, 8, 64], F32, tag="o2", name=f"o2_{i}_{phase}")
                nc.vector.scalar_tensor_tensor(
                    out=o2[:],
                    in0=psT3[:, :, 0:64],
                    scalar=1.0,
                    in1=recipv[:, :, None].to_broadcast([128, 8, 64]),
                    op0=MULT,
                    op1=MULT,
                )
                nc.sync.dma_start(
                    out[i].rearrange("(w t p) d -> w p t d", p=128, t=8)[phase],
                    o2[:],
                )


_CACHED = None


def _build():
    global _CACHED
    if _CACHED is not None:
        return _CACHED
    nc = bacc.Bacc("TRN2", target_bir_lowering=False, debug=False)
    qT = nc.dram_tensor("qT", [HPC, 128, L], BF16, kind="ExternalInput").ap()
    kT = nc.dram_tensor("kT", [HPC, 128, L], BF16, kind="ExternalInput").ap()
    v1 = nc.dram_tensor("v1", [HPC, 128, NCH * 65], BF16, kind="ExternalInput").ap()
    dlt = nc.dram_tensor("dlt", [HPC, 128, NCH], F32, kind="ExternalInput").ap()
    tau_in = nc.dram_tensor("tau_in", [HPC, 1], F32, kind="ExternalInput").ap()
    out = nc.dram_tensor("out", [HPC, L, E], F32, kind="ExternalOutput").ap()
    with tile.TileContext(nc) as tc:
        _body(tc, qT, kT, v1, dlt, tau_in, out)
    nc.compile()
    _CACHED = nc
    return nc


def _prep_in_maps(queries, keys, values, tau, delta):
    """Shard + relayout the full inputs into 8 per-core input dicts."""
    queries = np.asarray(queries, dtype=np.float32)
    keys = np.asarray(keys, dtype=np.float32)
    values = np.asarray(values, dtype=np.float32)
    tau = np.asarray(tau, dtype=np.float32)
    delta = np.asarray(delta, dtype=np.float32)

    in_maps = []
    for core in range(NCORES):
        qTs = np.zeros((HPC, 128, L), ml_dtypes.bfloat16)
        kTs = np.zeros((HPC, 128, L), ml_dtypes.bfloat16)
        v1s = np.empty((HPC, 128, NCH * 65), ml_dtypes.bfloat16)
        dls = np.empty((HPC, 128, NCH), np.float32)
        tas = np.empty((HPC, 1), np.float32)
        for slot in range(HPC):
            g = core * HPC + slot
            b, h = divmod(g, H)
            qTs[slot, 0:64] = queries[b, :, h, :].T
            kTs[slot, 0:64] = keys[b, :, h, :].T
            v = values[b, :, h, :].reshape(NCH, 128, E).transpose(1, 0, 2)
            vv = np.concatenate([v, np.ones((128, NCH, 1), np.float32)], axis=2)
            v1s[slot] = vv.reshape(128, NCH * 65).astype(ml_dtypes.bfloat16)
            dls[slot] = delta[b].reshape(NCH, 128).T
            tas[slot, 0] = tau[b, 0]
        in_maps.append(
            {"qT": qTs, "kT": kTs, "v1": v1s, "dlt": dls, "tau_in": tas}
        )
    return in_maps


def _assemble(results):
    O = np.empty((B, L, H, E), np.float32)
    for core in range(NCORES):
        o = results[core]["out"]  # [HPC, L, E]
        for slot in range(HPC):
            g = core * HPC + slot
            b, h = divmod(g, H)
            O[b, :, h, :] = o[slot]
    return O


def run(inputs, trace=False, **kwargs):
    from concourse import bass_utils

    nc = _build()
    in_maps = _prep_in_maps(**inputs)
    res = bass_utils.run_bass_kernel_spmd(
        nc, in_maps, core_ids=list(range(NCORES)), trace=trace, **kwargs
    )
    return _assemble(res.results), res


def kernel(**inputs):
    return run(inputs, trace=False)[0]


# revision 12
# speedup vs baseline: 1.5723x; 1.0159x over previous
"""DSAttention Trainium2 kernel.

Reference computation (per batch b, head h):
    S[q,s]  = (Q[q]·K[s]) * tau[b] + delta[b,s]
    S      += causal mask (s > q -> -inf)
    A       = softmax(S / sqrt(E), axis=s)
    O[q,:]  = sum_s A[q,s] * V[s,:]

Shapes: B=2, L=2048, H=16, E=64 -> 32 (b,h) pairs, 4 per NeuronCore x 8 cores.

Device strategy (per core, per head):
  - Q^T, K^T resident in SBUF as [e=64, L] (host pre-transposed).
  - V with a ones-column appended: [128, 16 chunks, 65]; the AV matmul then
    produces both O^T (rows 0..63) and the softmax denominator (row 64) in
    one PSUM accumulation.
  - Loop over s-chunks n (128 keys each): S^T[s,q] = K_n^T.T @ Q^T computed
    only for q >= 128n (causal skip), as <=512-wide matmul pieces into a
    [128,1024] PSUM tile (2 halves of the q range, double buffered).
  - One Exp activation per (chunk, half): exp(dot * (tau/8) + delta/8) with
    tau as a per-partition scale AP and delta/8 as a per-partition bias AP.
    No max-subtraction: |dot*tau/8 + delta/8| <= ~8, exp is safe in fp32.
  - Diagonal 128x128 block masked by multiplying with an upper-triangular
    0/1 matrix after the exp.
  - AV: O^T[65, q] += V1_n.T @ A^T accumulated over chunks in a [65, 2048]
    PSUM tile; when a 512-column q-tile is complete, divide rows 0..63 by
    row 64 (reciprocal + partition-broadcast DMA + scalar_tensor_tensor)
    and DMA out.  Output is O^T [64, L] per head; host transposes back.
  - Matmuls run in float32r (full-rate fp32 PE mode).
"""

import sys

sys.path.insert(0, "/opt/trn_rl_repo")

import ml_dtypes
import numpy as np

import concourse.bass as bass
import concourse.tile as tile
from concourse import bacc, mybir
from concourse.masks import make_identity, make_upper_triangular

B, L, H, E = 2, 2048, 16, 64
NCORES = 8
HPC = (B * H) // NCORES  # heads per core = 4
NCH = L // 128  # s-chunks per head = 16
SCALE = 1.0 / 8.0  # 1/sqrt(E)
F32 = mybir.dt.float32
F32R = mybir.dt.float32r
BF16 = mybir.dt.bfloat16
EXP = mybir.ActivationFunctionType.Exp
MULT = mybir.AluOpType.mult


def _pieces(n):
    """512-aligned matmul pieces covering q in [128n, L)."""
    q0 = 128 * n
    out = []
    c = q0
    w = 512 - 128 * (n % 4)
    out.append((c, w))
    c += w
    while c < L:
        out.append((c, 512))
        c += 512
    return out


def _body(tc, qT, kT, v1, tau_in, out):
    nc = tc.nc
    from contextlib import ExitStack

    with ExitStack() as ctx:
        const = ctx.enter_context(tc.tile_pool(name="const", bufs=1))
        qk_pool = ctx.enter_context(tc.tile_pool(name="qk", bufs=2))
        v_pool = ctx.enter_context(tc.tile_pool(name="v", bufs=2))
        hd_pool = ctx.enter_context(tc.tile_pool(name="hd", bufs=2))
        a_pool = ctx.enter_context(tc.tile_pool(name="a", bufs=3))
        o_pool = ctx.enter_context(tc.tile_pool(name="o", bufs=2))
        r_pool = ctx.enter_context(tc.tile_pool(name="r", bufs=2))
        ps_pool = ctx.enter_context(tc.tile_pool(name="psS", bufs=2, space="PSUM"))
        po_pool = ctx.enter_context(tc.tile_pool(name="psO", bufs=2, space="PSUM"))

        trimask = const.tile([128, 128], BF16, name="trimask")
        make_upper_triangular(nc, trimask[:], val=1.0, diag=True)
        ident65 = const.tile([65, 65], F32, name="ident65")
        make_identity(nc, ident65[:])

        for i in range(HPC):
            tt = hd_pool.tile([128, 1], F32, tag="tt", name=f"tt{i}")
            nc.sync.dma_start(tt[:], tau_in[i : i + 1, 0:1].to_broadcast([128, 1]))
            qt = qk_pool.tile([128, L], BF16, tag="qt", name=f"qt{i}")
            kt = qk_pool.tile([128, L], BF16, tag="kt", name=f"kt{i}")
            vt = v_pool.tile([128, NCH * 65], BF16, tag="vt", name=f"vt{i}")
            for hf in range(2):
                cs = slice(1024 * hf, 1024 * hf + 1024)
                nc.sync.dma_start(kt[:, cs], kT[i][:, cs])
                nc.sync.dma_start(qt[:, cs], qT[i][:, cs])
                vs = slice(8 * 65 * hf, 8 * 65 * hf + 8 * 65)
                nc.sync.dma_start(vt[:, vs], v1[i][:, vs])
                # fold tau into Q (rows 0..63 only; row 64 is the ones row
                # that delivers the delta bias from kt row 64)
                nc.vector.tensor_scalar_mul(
                    qt[0:64, cs], qt[0:64, cs], tt[0:64, 0:1]
                )

            # Two q-window phases per head: oT is [65, 1024] (2 PSUM
            # banks), leaving room to double-buffer both psumS and the
            # oT/psT slot.  AV matmuls are emitted two units behind their
            # QK/exp so the in-order PE queue never stalls on ACT.
            for phase in range(2):
                qlo = 1024 * phase
                qhi = qlo + 1024
                oT = po_pool.tile([65, 1024], F32, tag="oT", name=f"oT{i}_{phase}")
                units = []  # (n, pieces, a_sb)
                pend = []   # emitted QK/exp awaiting AV emission
                nlist = list(range(qhi // 128))

                def emit_av(u):
                    n, pieces, a_sb = u
                    for c0, w in pieces:
                        b = (c0 - qlo) // 512
                        j = 2 * phase + b
                        nc.tensor.matmul(
                            oT[:, c0 - qlo : c0 - qlo + w],
                            lhsT=vt[:, n * 65 : n * 65 + 65],
                            rhs=a_sb[:, c0 - qlo : c0 - qlo + w],
                            start=(n == 0),
                            stop=(n == 4 * j + 3),
                        )

                for n in nlist:
                    q0 = max(128 * n, qlo)
                    pieces = []
                    c = q0
                    w0 = 512 * (q0 // 512 + 1) - q0
                    pieces.append((c, w0))
                    c += w0
                    while c < qhi:
                        pieces.append((c, 512))
                        c += 512
                    ps = ps_pool.tile([128, 1024], F32, tag="ps", name=f"ps{i}_{phase}_{n}")
                    for c0, w in pieces:
                        nc.tensor.matmul(
                            ps[:, c0 - qlo : c0 - qlo + w],
                            lhsT=kt[:, 128 * n : 128 * n + 128],
                            rhs=qt[:, c0 : c0 + w],
                            start=True,
                            stop=True,
                        )
                    a_sb = a_pool.tile([128, 1024], BF16, tag="a", name=f"a{i}_{phase}_{n}")
                    nc.scalar.activation(
                        a_sb[:, q0 - qlo : 1024],
                        ps[:, q0 - qlo : 1024],
                        EXP,
                        scale=SCALE,
                    )
                    if 128 * n >= qlo:
                        nc.vector.tensor_mul(
                            a_sb[:, q0 - qlo : q0 - qlo + 128],
                            a_sb[:, q0 - qlo : q0 - qlo + 128],
                            trimask[:],
                        )
                    pend.append((n, pieces, a_sb))
                    if len(pend) > 2:
                        emit_av(pend.pop(0))
                for u in pend:
                    emit_av(u)

                # Window finalization: PSUM -> SBUF, PE-transpose each 128-q
                # chunk, reciprocal of the denominator row, scale, store.
                o_sb = o_pool.tile([65, 1024], F32, tag="osb", name=f"osb{i}_{phase}")
                nc.vector.tensor_copy(o_sb[:], oT[:, :])
                psT = po_pool.tile([128, 1024], F32, tag="oT", name=f"psT{i}_{phase}")
                for t in range(8):
                    nc.tensor.transpose(
                        psT[:, 128 * t : 128 * t + 65],
                        o_sb[:, 128 * t : 128 * t + 128],
                        ident65[:],
                    )
                psT3 = psT.rearrange("p (t c) -> p t c", c=128)
                recipv = r_pool.tile([128, 8], F32, tag="rr", name=f"rr{i}_{phase}")
                nc.vector.reciprocal(recipv[:], psT3[:, :, 64])
                o2 = o_pool.tile([128, 8, 64], F32, tag="o2", name=f"o2_{i}_{phase}")
                nc.vector.scalar_tensor_tensor(
                    out=o2[:],
                    in0=psT3[:, :, 0:64],
                    scalar=1.0,
                    in1=recipv[:, :, None].to_broadcast([128, 8, 64]),
                    op0=MULT,
                    op1=MULT,
                )
                nc.sync.dma_start(
                    out[i].rearrange("(w t p) d -> w p t d", p=128, t=8)[phase],
                    o2[:],
                )


_CACHED = None


def _build():
    global _CACHED
    if _CACHED is not None:
        return _CACHED
    nc = bacc.Bacc("TRN2", target_bir_lowering=False, debug=False)
    qT = nc.dram_tensor("qT", [HPC, 128, L], BF16, kind="ExternalInput").ap()
    kT = nc.dram_tensor("kT", [HPC, 128, L], BF16, kind="ExternalInput").ap()
    v1 = nc.dram_tensor("v1", [HPC, 128, NCH * 65], BF16, kind="ExternalInput").ap()
    tau_in = nc.dram_tensor("tau_in", [HPC, 1], F32, kind="ExternalInput").ap()
    out = nc.dram_tensor("out", [HPC, L, E], F32, kind="ExternalOutput").ap()
    with tile.TileContext(nc) as tc:
        _body(tc, qT, kT, v1, tau_in, out)
    nc.compile()
    _CACHED = nc
    return nc


def _prep_in_maps(queries, keys, values, tau, delta):
    """Shard + relayout the full inputs into 8 per-core input dicts."""
    queries = np.asarray(queries, dtype=np.float32)
    keys = np.asarray(keys, dtype=np.float32)
    values = np.asarray(values, dtype=np.float32)
    tau = np.asarray(tau, dtype=np.float32)
    delta = np.asarray(delta, dtype=np.float32)

    in_maps = []
    for core in range(NCORES):
        qTs = np.zeros((HPC, 128, L), ml_dtypes.bfloat16)
        kTs = np.zeros((HPC, 128, L), ml_dtypes.bfloat16)
        v1s = np.empty((HPC, 128, NCH * 65), ml_dtypes.bfloat16)
        tas = np.empty((HPC, 1), np.float32)
        for slot in range(HPC):
            g = core * HPC + slot
            b, h = divmod(g, H)
            qTs[slot, 0:64] = queries[b, :, h, :].T
            qTs[slot, 64, :] = 1.0
            kTs[slot, 0:64] = keys[b, :, h, :].T
            kTs[slot, 64, :] = delta[b, :]
            v = values[b, :, h, :].reshape(NCH, 128, E).transpose(1, 0, 2)
            vv = np.concatenate([v, np.ones((128, NCH, 1), np.float32)], axis=2)
            v1s[slot] = vv.reshape(128, NCH * 65).astype(ml_dtypes.bfloat16)
            tas[slot, 0] = tau[b, 0]
        in_maps.append({"qT": qTs, "kT": kTs, "v1": v1s, "tau_in": tas})
    return in_maps


def _assemble(results):
    O = np.empty((B, L, H, E), np.float32)
    for core in range(NCORES):
        o = results[core]["out"]  # [HPC, L, E]
        for slot in range(HPC):
            g = core * HPC + slot
            b, h = divmod(g, H)
            O[b, :, h, :] = o[slot]
    return O


def run(inputs, trace=False, **kwargs):
    from concourse import bass_utils

    nc = _build()
    in_maps = _prep_in_maps(**inputs)
    res = bass_utils.run_bass_kernel_spmd(
        nc, in_maps, core_ids=list(range(NCORES)), trace=trace, **kwargs
    )
    return _assemble(res.results), res


def kernel(**inputs):
    return run(inputs, trace=False)[0]
